# revision 28
# baseline (speedup 1.0000x reference)
"""Trainium2 Bass kernel for nn_EdgeDecoder (GNN edge decoder, 2 relations).

Strategy (data-parallel over edges, 8 NeuronCores):
  - Shard the 500k edges of each relation across 8 cores (62500/core).
  - Host pre-gathers node embeddings into per-core, per-relation tables in
    DIM-MAJOR edge order: [128(dim), EPAD(edge)] fp16 (EPAD = 62592).  Device
    reads are then plain contiguous 2D DMA streams (16KB/partition bursts):
    no SWDGE descriptors, no index upload; outputs return in edge order.
  - |W2| is folded into W1's columns on host (relu(x)*|w2| == relu(x*|w2|)),
    and columns are permuted positives-first.  The edge logit becomes
    sum(pos cols of relu) - sum(neg cols of relu): a plain segmented
    reduction, no per-edge multiply on device.
  - Device, per chunk of 8192 edges: for each 128-edge block,
        psum[e, h] = gu_blk^T @ W1u' + gv_blk^T @ W1v'   (2 matmuls, N=256)
    Per 4-block PSUM quad (2 banks), one relu evacuation to fp16 SBUF
    (scalar engine mostly, some on DVE as tensor_tensor max).  Per 8-block
    group, two segmented tensor_reduce instructions (pos cols / neg cols)
    produce per-edge partial sums; host computes pos - neg.  Every
    H_EVERY'th group instead DMAs the fp16 activations out and the host
    does that dot (keeps DVE under the matmul/DMA roofline).
"""
import sys

if "/opt/trn_rl_repo" not in sys.path:
    sys.path.insert(0, "/opt/trn_rl_repo")

import numpy as np

P = 128
D = 128
HID = 256
E = 500000
NCORES = 8
EPC = E // NCORES            # 62500 edges per core per relation
EPAD = 62592                 # padded to a multiple of 128 (489 blocks)
CH = 8192                    # edges per DMA chunk
NCH = -(-EPAD // CH)         # 8 chunks: 7 full + 1 of 5248
CHUNK_LENS = [min(CH, EPAD - c * CH) for c in range(NCH)]
MAXBLK = CH // P             # 64 block-columns in the output tiles
NREL = 2
GRP = 8                      # blocks per reduce group
H_PATTERN = (False, False, True, False, False, True, False, True)
DVE_EVAC_EVERY = 16          # every n'th quad evacuates on DVE, not Act

_PROGRAM_CACHE = {}
LAST_RESULTS = None


def _schedule():
    """Yields (rel, chunk, jb, g, gidx, is_host, hrow) group descriptors.

    hrow counts host groups per relation (row index into oht{r})."""
    out = []
    gidx = 0
    hrow = [0, 0]
    for r in range(NREL):
        for c in range(NCH):
            nblk = CHUNK_LENS[c] // P
            jb = 0
            while jb < nblk:
                g = min(GRP, nblk - jb)
                is_host = H_PATTERN[gidx % len(H_PATTERN)]
                out.append((r, c, jb, g, gidx, is_host,
                            hrow[r] if is_host else -1))
                if is_host:
                    hrow[r] += 1
                gidx += 1
                jb += g
    return out, hrow


SCHED, NHG = _schedule()


def _build_program(nonzero_b1, npos):
    import concourse.bacc as bacc
    import concourse.mybir as mybir
    from concourse.tile import TileContext

    f16, f32 = mybir.dt.float16, mybir.dt.float32
    Alu = mybir.AluOpType
    Relu = mybir.ActivationFunctionType.Relu
    X = mybir.AxisListType.X

    nc = bacc.Bacc("TRN2", target_bir_lowering=False, debug=False)

    ut_d, vt_d, w1u_d, w1v_d, b1_d = {}, {}, {}, {}, {}
    lp_d, ln_d, oht_d = {}, {}, {}
    for r in range(NREL):
        ut_d[r] = nc.dram_tensor(f"utT{r}", [P, EPAD], f16, kind="ExternalInput")
        vt_d[r] = nc.dram_tensor(f"vtT{r}", [P, EPAD], f16, kind="ExternalInput")
        w1u_d[r] = nc.dram_tensor(f"w1u{r}", [D, HID], f16, kind="ExternalInput")
        w1v_d[r] = nc.dram_tensor(f"w1v{r}", [D, HID], f16, kind="ExternalInput")
        if nonzero_b1:
            b1_d[r] = nc.dram_tensor(f"b1r{r}", [P, GRP * HID], f32,
                                     kind="ExternalInput")
        lp_d[r] = nc.dram_tensor(f"lp{r}", [NCH, P, MAXBLK], f32,
                                 kind="ExternalOutput")
        ln_d[r] = nc.dram_tensor(f"ln{r}", [NCH, P, MAXBLK], f32,
                                 kind="ExternalOutput")
        oht_d[r] = nc.dram_tensor(f"oht{r}", [max(NHG[r], 1), P, GRP * HID],
                                  f16, kind="ExternalOutput")

    by_chunk = {}
    for (r, c, jb, g, gidx, is_host, hrow) in SCHED:
        by_chunk.setdefault((r, c), []).append((jb, g, gidx, is_host, hrow))

    with TileContext(nc) as tc:
        with tc.tile_pool(name="sbw", bufs=1) as sbw, \
             tc.tile_pool(name="sbg", bufs=3) as sbg, \
             tc.tile_pool(name="sbh", bufs=4) as sbh, \
             tc.tile_pool(name="sblog", bufs=3) as sblog, \
             tc.tile_pool(name="ppB", bufs=4, space="PSUM") as ppB:

            w1u_t, w1v_t, b1_t = [], [], []
            for r in range(NREL):
                t = sbw.tile([D, HID], f16, tag=f"w1u{r}")
                nc.sync.dma_start(out=t[:], in_=w1u_d[r].ap()[:])
                w1u_t.append(t)
                t = sbw.tile([D, HID], f16, tag=f"w1v{r}")
                nc.sync.dma_start(out=t[:], in_=w1v_d[r].ap()[:])
                w1v_t.append(t)
                if nonzero_b1:
                    t = sbw.tile([P, GRP * HID], f32, tag=f"b1{r}")
                    nc.sync.dma_start(out=t[:], in_=b1_d[r].ap()[:])
                    b1_t.append(t)
            zero8 = sbw.tile([P, GRP * HID], f16, tag="zero8")
            nc.vector.memset(zero8[:], 0.0)

            qctr = 0
            for r in range(NREL):
                for c in range(NCH):
                    L = CHUNK_LENS[c]
                    nblk = L // P
                    gu = sbg.tile([P, CH], f16, tag="gu")
                    gv = sbg.tile([P, CH], f16, tag="gv")
                    if r == 0 and c == 0:
                        # slice the very first loads so compute starts early
                        q4 = L // 4
                        for s in range(4):
                            lo, hi = s * q4, (s + 1) * q4 if s < 3 else L
                            nc.sync.dma_start(
                                out=gu[:, lo:hi],
                                in_=ut_d[r].ap()[:, c * CH + lo:c * CH + hi])
                            nc.sync.dma_start(
                                out=gv[:, lo:hi],
                                in_=vt_d[r].ap()[:, c * CH + lo:c * CH + hi])
                    else:
                        nc.sync.dma_start(
                            out=gu[:, :L],
                            in_=ut_d[r].ap()[:, c * CH:c * CH + L])
                        nc.sync.dma_start(
                            out=gv[:, :L],
                            in_=vt_d[r].ap()[:, c * CH:c * CH + L])
                    lp = sblog.tile([P, MAXBLK], f32, tag="lp")
                    ln = sblog.tile([P, MAXBLK], f32, tag="ln")
                    any_pos = False
                    any_neg = False
                    for (jb, g, gidx, is_host, hrow) in by_chunk[(r, c)]:
                        nq = -(-g // 4)
                        psq = []
                        for q in range(nq):
                            pq = ppB.tile([P, 4 * HID], f32, tag="ppB",
                                          name=f"pq{gidx}_{q}")
                            psq.append(pq)
                        for t in range(g):
                            e0 = (jb + t) * P
                            sl = psq[t // 4][:, (t % 4) * HID:
                                             (t % 4) * HID + HID]
                            nc.tensor.matmul(out=sl, lhsT=gu[:, e0:e0 + P],
                                             rhs=w1u_t[r][:],
                                             start=True, stop=False)
                            nc.tensor.matmul(out=sl, lhsT=gv[:, e0:e0 + P],
                                             rhs=w1v_t[r][:],
                                             start=False, stop=True)
                        ht8 = sbh.tile([P, GRP * HID], f16, tag="ht8")
                        for q in range(nq):
                            w = min(4, g - q * 4) * HID
                            if nonzero_b1:
                                nc.vector.tensor_tensor(
                                    out=psq[q][:, :w], in0=psq[q][:, :w],
                                    in1=b1_t[r][:, :w], op=Alu.add)
                            osl = ht8[:, q * 4 * HID:q * 4 * HID + w]
                            if qctr % DVE_EVAC_EVERY == DVE_EVAC_EVERY - 1:
                                nc.vector.tensor_tensor(
                                    out=osl, in0=psq[q][:, :w],
                                    in1=zero8[:, :w], op=Alu.max)
                            else:
                                nc.scalar.activation(out=osl,
                                                     in_=psq[q][:, :w],
                                                     func=Relu)
                            qctr += 1
                        if is_host:
                            nc.sync.dma_start(
                                out=oht_d[r].ap()[hrow][:, :g * HID],
                                in_=ht8[:, :g * HID])
                        else:
                            h3 = ht8[:, :g * HID].rearrange(
                                "p (b h) -> p b h", h=HID)
                            if npos[r] > 0:
                                nc.vector.tensor_reduce(
                                    out=lp[:, jb:jb + g],
                                    in_=h3[:, :, 0:npos[r]],
                                    axis=X, op=Alu.add)
                                any_pos = True
                            if npos[r] < HID:
                                nc.vector.tensor_reduce(
                                    out=ln[:, jb:jb + g],
                                    in_=h3[:, :, npos[r]:HID],
                                    axis=X, op=Alu.add)
                                any_neg = True
                    if any_pos:
                        nc.sync.dma_start(out=lp_d[r].ap()[c][:, :nblk],
                                          in_=lp[:, :nblk])
                    if any_neg:
                        nc.sync.dma_start(out=ln_d[r].ap()[c][:, :nblk],
                                          in_=ln[:, :nblk])
    nc.compile()
    return nc


def _prep(user_embed, item_embed, u_clicks, v_clicks, u_buys, v_buys,
          W1_clicks, b1_clicks, W2_clicks, b2_clicks,
          W1_buys, b1_buys, W2_buys, b2_buys):
    user16 = np.asarray(user_embed, np.float32).astype(np.float16)
    item16 = np.asarray(item_embed, np.float32).astype(np.float16)
    rels = [
        (np.asarray(u_clicks), np.asarray(v_clicks),
         np.asarray(W1_clicks, np.float32), np.asarray(b1_clicks, np.float32),
         np.asarray(W2_clicks, np.float32), np.asarray(b2_clicks, np.float32)),
        (np.asarray(u_buys), np.asarray(v_buys),
         np.asarray(W1_buys, np.float32), np.asarray(b1_buys, np.float32),
         np.asarray(W2_buys, np.float32), np.asarray(b2_buys, np.float32)),
    ]
    nonzero_b1 = any(np.any(rels[r][3] != 0.0) for r in range(NREL))
    b2_host = [float(rels[r][5][0]) for r in range(NREL)]

    # fold |w2| into W1 columns; permute positives first
    npos, perms, sgn_neg, w2_perm = [], [], [], []
    folded = []
    for r in range(NREL):
        W1, b1, W2 = rels[r][2], rels[r][3], rels[r][4]
        pos = np.where(W2 >= 0)[0]
        neg = np.where(W2 < 0)[0]
        perm = np.concatenate([pos, neg])
        npos.append(len(pos))
        perms.append(perm)
        scale = np.abs(W2)[perm]
        w1p = (W1[:, perm] * scale[None, :]).astype(np.float16)
        b1p = (b1[perm] * scale).astype(np.float32)
        # oht holds |w2|-folded activations; host applies only the signs
        w2_perm.append(np.where(np.arange(HID) < len(pos), 1.0, -1.0)
                       .astype(np.float32))
        folded.append((w1p[:D], w1p[D:], b1p))

    in_maps = []
    for k in range(NCORES):
        m = {}
        for r in range(NREL):
            u_all, v_all = rels[r][0], rels[r][1]
            w1u, w1v, b1p = folded[r]
            m[f"w1u{r}"] = w1u
            m[f"w1v{r}"] = w1v
            if nonzero_b1:
                m[f"b1r{r}"] = np.tile(b1p[None, :], (P, GRP))
            lo = k * EPC
            ut = np.zeros((EPAD, D), np.float16)
            ut[:EPC] = user16[u_all[lo:lo + EPC]]
            m[f"utT{r}"] = np.ascontiguousarray(ut.T)
            vt = np.zeros((EPAD, D), np.float16)
            vt[:EPC] = item16[v_all[lo:lo + EPC]]
            m[f"vtT{r}"] = np.ascontiguousarray(vt.T)
        in_maps.append(m)
    return nonzero_b1, tuple(npos), w2_perm, b2_host, in_maps


def kernel(**inputs):
    global LAST_RESULTS
    from concourse import bass_utils

    nonzero_b1, npos, w2_perm, b2_host, in_maps = _prep(**inputs)

    key = (nonzero_b1, npos)
    if key not in _PROGRAM_CACHE:
        _PROGRAM_CACHE[key] = _build_program(nonzero_b1, npos)
    nc = _PROGRAM_CACHE[key]

    res = bass_utils.run_bass_kernel_spmd(nc, in_maps,
                                          core_ids=list(range(NCORES)))
    LAST_RESULTS = res

    outs = []
    for r in range(NREL):
        full = np.empty(E, np.float32)
        for k in range(NCORES):
            rk = res.results[k]
            lp = rk[f"lp{r}"].astype(np.float32)
            ln = rk[f"ln{r}"].astype(np.float32)
            if npos[r] == 0:
                log = -ln
            elif npos[r] == HID:
                log = lp
            else:
                log = lp - ln                       # [NCH, P, MAXBLK]
            if NHG[r] > 0:
                oht = rk[f"oht{r}"]                 # [NHG, P, GRP*HID]
                hsel = [(c, jb, g, hrow) for (rr, c, jb, g, _gi, ih, hrow)
                        in SCHED if ih and rr == r]
                hstack = oht.reshape(NHG[r], P, GRP, HID).astype(np.float32)
                red = hstack @ w2_perm[r]           # [NHG, P, GRP]
                for (c, jb, g, hrow) in hsel:
                    log[c, :, jb:jb + g] = red[hrow, :, :g]
            lin = log.transpose(0, 2, 1).reshape(-1)[:EPC]
            full[k * EPC:(k + 1) * EPC] = lin
        if b2_host[r] != 0.0:
            full += b2_host[r]
        outs.append(full)
    return outs[0], outs[1]
